# revision 4
# baseline (speedup 1.0000x reference)
"""GNN message passing (nn_Net_16329465660089) as a Bass/Tile kernel on 8 trn2 cores.

Math:  We[e] = efeat[e] * W0 + B0  (rank-1 structure of the edge MLP), so
    msgs[e] = h_src[e] @ We[e] = efeat[e] * (h_src[e] @ W0) + h_src[e] @ B0
    out     = relu(segment_mean(msgs, dst) + bias)

Sharding: edges are routed to the core that owns their destination node
(core k owns nodes [1250k, 1250(k+1))), so each core computes its output
slice with no collectives.  Per core: gather h_src rows by indirect DMA,
transpose on PE, two shared matmuls + per-edge scale for the messages,
one-hot matmul for the segment sum (+ count), fused mean/bias/relu epilogue.
The relu(mean + bias) is computed as relu((sums + max(cnt,1)*bias) * (1/max(cnt,1)))
which is exact for cnt==0 nodes too.
"""

import re
import numpy as np
from contextlib import ExitStack

import bass_rust
import concourse.bass as bass
import concourse.mybir as mybir
import concourse.tile as tile
from concourse.bass_utils import run_bass_kernel_spmd

F32 = mybir.dt.float32
I32 = mybir.dt.int32

N_NODES = 10000
N_EDGES = 4096
IN_F = 300
OUT_F = 150
NCORES = 8
NPC = N_NODES // NCORES          # nodes per core (1250)
P = 128
NM = (NPC + P - 1) // P          # node chunks per core (10)
KCH = [(0, 128), (128, 128), (256, IN_F - 256)]  # contraction chunks of IN_F

LAST_RESULTS = None              # test harness reads exec_time from here


def _patch_tile_drain():
    """The stock Tile tail attaches every outstanding sem wait to one SP Drain
    instruction; this walrus build rejects multi-wait Drains ("Too many sync
    wait commands").  Split the waits onto individual SP nops instead."""
    if getattr(tile.TileContext, "_drain_split_patch", False):
        return

    def _drain_and_barrier(self, tick_clock, wait_clock):
        gc = tick_clock.global_clock
        vals = [int(x) for x in re.findall(r"-?\d+", repr(gc))]
        n = len(vals)
        for i, v in enumerate(vals):
            if v > 0:
                sub = bass_rust.VectorClock([v if j == i else 0 for j in range(n)])
                nop = self.nc.sync.nop()
                wait_clock.add_sem_waits(nop.ins, tile.ScopedClock({None: sub}))
        self.nc.sync.drain()
        self.nc.all_engine_barrier()
        popped = self.nc._tile_sem_poison_stack.pop()
        assert popped is self._sem_poison
        self.nc.clear_and_free_semaphores(list(self.sems.allocated().values()))
        self.nc.all_engine_barrier()

    tile.TileContext._drain_and_barrier = _drain_and_barrier
    tile.TileContext._drain_split_patch = True


def _build_program(C):
    """Emit the per-core Bass program for edge capacity C (multiple of 128)."""
    _patch_tile_drain()
    NE = C // P                  # edge chunks

    nc = bass.Bass()
    feats_d = nc.dram_tensor("features", [N_NODES, IN_F], F32, kind="ExternalInput")
    srcidx_d = nc.dram_tensor("srcidx", [C, 1], I32, kind="ExternalInput")
    dstf_d = nc.dram_tensor("dstf", [C], F32, kind="ExternalInput")
    eff_d = nc.dram_tensor("eff", [C], F32, kind="ExternalInput")
    wb_d = nc.dram_tensor("wb", [2 * IN_F, OUT_F], F32, kind="ExternalInput")
    bias_d = nc.dram_tensor("bias1", [1, OUT_F], F32, kind="ExternalInput")
    ident_d = nc.dram_tensor("ident", [P, P], F32, kind="ExternalInput")
    iota_d = nc.dram_tensor("iota", [P, NPC], F32, kind="ExternalInput")
    out_d = nc.dram_tensor("out", [NPC, OUT_F], F32, kind="ExternalOutput")

    with tile.TileContext(nc) as tc, ExitStack() as ctx:
        const = ctx.enter_context(tc.tile_pool(name="const", bufs=1))
        gath = ctx.enter_context(tc.tile_pool(name="gath", bufs=3))
        big = ctx.enter_context(tc.tile_pool(name="big", bufs=1))
        small = ctx.enter_context(tc.tile_pool(name="small", bufs=4))
        outp = ctx.enter_context(tc.tile_pool(name="outp", bufs=3))
        pt = ctx.enter_context(tc.tile_pool(name="pt", bufs=2, space="PSUM"))
        pq = ctx.enter_context(tc.tile_pool(name="pq", bufs=2, space="PSUM"))
        psums = ctx.enter_context(tc.tile_pool(name="psums", bufs=2, space="PSUM"))

        # ---- constants / small loads ----
        ident = const.tile([P, P], F32)
        nc.sync.dma_start(ident[:], ident_d[:])
        iota_w = const.tile([P, NPC], F32)
        nc.sync.dma_start(iota_w[:], iota_d[:])
        bias_row = const.tile([1, OUT_F], F32)
        nc.sync.dma_start(bias_row[:], bias_d[:])
        dst_sb = const.tile([P, NE], F32)
        nc.sync.dma_start(dst_sb[:], dstf_d.rearrange("(c p) -> p c", p=P))
        eff_sb = const.tile([P, NE], F32)
        nc.sync.dma_start(eff_sb[:], eff_d.rearrange("(c p) -> p c", p=P))
        wbk = []
        for j, (o, w) in enumerate(KCH):
            t0 = const.tile([w, OUT_F], F32, name=f"w0k{j}", tag=f"w0k{j}")
            nc.sync.dma_start(t0[:], wb_d[o:o + w, :])
            t1 = const.tile([w, OUT_F], F32, name=f"b0k{j}", tag=f"b0k{j}")
            nc.sync.dma_start(t1[:], wb_d[IN_F + o:IN_F + o + w, :])
            wbk.append((t0, t1))

        # ---- gather + transpose h_src ----
        hsT = [big.tile([w, C], F32, name=f"hsT{j}", tag=f"hsT{j}")
               for j, (o, w) in enumerate(KCH)]
        for e in range(NE):
            si = gath.tile([P, 1], I32, tag="si")
            nc.sync.dma_start(si[:], srcidx_d[e * P:(e + 1) * P, :])
            hs = gath.tile([P, IN_F], F32, tag="hs")
            nc.gpsimd.indirect_dma_start(
                out=hs[:],
                out_offset=None,
                in_=feats_d[:],
                in_offset=bass.IndirectOffsetOnAxis(ap=si[:, :1], axis=0),
            )
            for j, (o, w) in enumerate(KCH):
                tp = pt.tile([w, P], F32, tag="tp")
                nc.tensor.transpose(out=tp[:], in_=hs[:, o:o + w], identity=ident[:])
                nc.scalar.copy(out=hsT[j][:, e * P:(e + 1) * P], in_=tp[:])

        # ---- one-hot selection matrices (edges x local-node), DVE/GPSIMD split ----
        s_full = []
        for e in range(NE):
            st = big.tile([P, NPC], F32, name=f"sfull{e}", tag=f"sfull{e}")
            eng = nc.vector if e % 2 == 0 else nc.gpsimd
            eng.tensor_scalar(
                out=st[:], in0=iota_w[:],
                scalar1=dst_sb[:, e:e + 1], scalar2=None,
                op0=mybir.AluOpType.is_equal,
            )
            s_full.append(st)

        # ---- messages: msgs = eff * (h @ W0) + h @ B0, plus ones column ----
        msgs = []
        for e in range(NE):
            pp = pq.tile([P, OUT_F], F32, tag="pp")
            qp = pq.tile([P, OUT_F], F32, tag="qp")
            for j, (o, w) in enumerate(KCH):
                nc.tensor.matmul(pp[:], lhsT=hsT[j][:, e * P:(e + 1) * P],
                                 rhs=wbk[j][0][:], start=(j == 0), stop=(j == 2))
            for j, (o, w) in enumerate(KCH):
                nc.tensor.matmul(qp[:], lhsT=hsT[j][:, e * P:(e + 1) * P],
                                 rhs=wbk[j][1][:], start=(j == 0), stop=(j == 2))
            mt = big.tile([P, OUT_F + 1], F32, name=f"msgs{e}", tag=f"msgs{e}")
            tmp = outp.tile([P, OUT_F], F32, tag="ptmp")
            nc.scalar.activation(tmp[:], pp[:], mybir.ActivationFunctionType.Copy,
                                 scale=eff_sb[:, e:e + 1])
            nc.vector.tensor_tensor(out=mt[:, 0:OUT_F], in0=tmp[:], in1=qp[:],
                                    op=mybir.AluOpType.add)
            nc.gpsimd.memset(mt[:, OUT_F:OUT_F + 1], 1.0)
            msgs.append(mt)

        # ---- segment sum + mean/bias/relu epilogue, one node chunk at a time ----
        for m in range(NM):
            mo = m * P
            mw = min(P, NPC - mo)
            sp = psums.tile([mw, OUT_F + 1], F32, tag="sp")
            for e in range(NE):
                nc.tensor.matmul(sp[:], lhsT=s_full[e][:, mo:mo + mw], rhs=msgs[e][:],
                                 start=(e == 0), stop=(e == NE - 1))
            cntm = small.tile([mw, 1], F32, tag="cntm")
            nc.vector.tensor_scalar_max(cntm[:], sp[:, OUT_F:OUT_F + 1], 1.0)
            recip = small.tile([mw, 1], F32, tag="recip")
            nc.vector.reciprocal(recip[:], cntm[:])
            ctp = pt.tile([1, mw], F32, tag="tp")
            nc.tensor.transpose(out=ctp[:], in_=cntm[:], identity=ident[0:mw, 0:mw])
            crow = small.tile([1, mw], F32, tag="crow")
            nc.scalar.copy(out=crow[:], in_=ctp[:])
            # sums += max(cnt,1) x bias  (rank-1), then relu(sums * 1/max(cnt,1))
            nc.tensor.matmul(sp[:, 0:OUT_F], lhsT=crow[:], rhs=bias_row[:],
                             start=False, stop=True, skip_group_check=True)
            ob = outp.tile([mw, OUT_F], F32, tag="ob")
            nc.scalar.activation(ob[:], sp[:, 0:OUT_F],
                                 mybir.ActivationFunctionType.Relu, scale=recip[:])
            nc.sync.dma_start(out_d[mo:mo + mw, :], ob[:])

    return nc


_MAX_WAITS = 2
_MAX_UPDATES = 2


def _split_syncs(nc):
    """This walrus build encodes only a couple of sem wait/update slots per
    instruction.  Hoist excess waits onto preceding same-engine NoOps and
    defer excess updates onto trailing same-engine NoOps."""
    n = 0
    for f in nc.m.functions:
        for bb in f.blocks:
            out = []
            for inst in bb.instructions:
                si = getattr(inst, "sync_info", None)
                waits = list(si.on_wait) if si is not None and si.on_wait else []
                updates = list(si.on_update) if si is not None and si.on_update else []
                pre, post = [], []
                if len(waits) > _MAX_WAITS:
                    excess, waits = waits[:-_MAX_WAITS], waits[-_MAX_WAITS:]
                    for i in range(0, len(excess), _MAX_WAITS):
                        n += 1
                        pre.append(mybir.InstNoOp(
                            name=f"I-wsplit-{n}", engine=inst.engine, ins=[], outs=[],
                            sync_info=mybir.SyncInfo(
                                on_wait=excess[i:i + _MAX_WAITS], on_update=[]),
                        ))
                if len(updates) > _MAX_UPDATES:
                    updates, excess = updates[:_MAX_UPDATES], updates[_MAX_UPDATES:]
                    for i in range(0, len(excess), _MAX_UPDATES):
                        n += 1
                        post.append(mybir.InstNoOp(
                            name=f"I-usplit-{n}", engine=inst.engine, ins=[], outs=[],
                            sync_info=mybir.SyncInfo(
                                on_wait=[], on_update=excess[i:i + _MAX_UPDATES]),
                        ))
                if (pre or post) and si is not None:
                    inst.sync_info = mybir.SyncInfo(on_wait=waits, on_update=updates)
                out.extend(pre)
                out.append(inst)
                out.extend(post)
            bb.instructions[:] = out


_PROGRAM_CACHE = {}


def _get_program(C):
    if C not in _PROGRAM_CACHE:
        nc = _build_program(C)
        _split_syncs(nc)
        _PROGRAM_CACHE[C] = nc
    return _PROGRAM_CACHE[C]


def kernel(features, efeat, W_edge, b_edge, bias, src, dst):
    global LAST_RESULTS
    features = np.ascontiguousarray(np.asarray(features, np.float32))
    efeat = np.asarray(efeat, np.float32).reshape(-1)
    bias = np.asarray(bias, np.float32).reshape(-1)
    src = np.asarray(src, np.int32).reshape(-1)
    dst = np.asarray(dst, np.int32).reshape(-1)
    wb = np.concatenate(
        [np.asarray(W_edge, np.float32).reshape(IN_F, OUT_F),
         np.asarray(b_edge, np.float32).reshape(IN_F, OUT_F)], axis=0)
    wb = np.ascontiguousarray(wb)

    core_of = dst // NPC
    sel = [np.nonzero(core_of == k)[0] for k in range(NCORES)]
    maxn = max(len(s) for s in sel)
    C = max(768, -(-maxn // P) * P)

    nc = _get_program(C)

    ident = np.eye(P, dtype=np.float32)
    iota = np.tile(np.arange(NPC, dtype=np.float32), (P, 1))
    bias1 = np.ascontiguousarray(bias[None, :])

    in_maps = []
    for k in range(NCORES):
        s = sel[k]
        n = len(s)
        si = np.zeros((C, 1), np.int32)
        si[:n, 0] = src[s]
        df = np.full((C,), -1.0, np.float32)
        df[:n] = (dst[s] - NPC * k).astype(np.float32)
        ef = np.zeros((C,), np.float32)
        ef[:n] = efeat[s]
        in_maps.append(dict(features=features, srcidx=si, dstf=df, eff=ef,
                            wb=wb, bias1=bias1, ident=ident, iota=iota))

    res = run_bass_kernel_spmd(nc, in_maps, core_ids=list(range(NCORES)))
    LAST_RESULTS = res
    return np.concatenate([res.results[k]["out"] for k in range(NCORES)], axis=0)


# revision 8
# speedup vs baseline: 1.9318x; 1.9318x over previous
"""GNN message passing (nn_Net_16329465660089) as a Bass/Tile kernel on 8 trn2 cores.

Math:  We[e] = efeat[e] * W0 + B0  (rank-1 structure of the edge MLP), so
    msgs[e] = h_src[e] @ We[e] = efeat[e] * (h_src[e] @ W0) + h_src[e] @ B0
    out     = relu(segment_mean(msgs, dst) + bias)

Sharding: edges are routed to the core that owns their destination node
(core k owns nodes [1250k, 1250(k+1))), so each core computes its output
slice with no collectives.  Per core: gather h_src rows by indirect DMA,
hi/lo bf16 split + transpose on PE, shared matmuls + per-edge scale for the
messages, one-hot matmul for the segment sum (+ count), fused
mean/bias/relu epilogue.  All matmuls run in bf16 with hi/lo error
compensation (h = hh + hl, W = Whi + Wlo, msgs = mhi + mlo), which keeps
the overall relative error at ~1e-5 while using full-rate bf16 PE passes.
The relu(mean + bias) is computed as
relu((sums + max(cnt,1)*bias) * (1/max(cnt,1))), exact for cnt==0 nodes.
"""

import re
import numpy as np
from contextlib import ExitStack

import ml_dtypes
import bass_rust
import concourse.bass as bass
import concourse.mybir as mybir
import concourse.tile as tile
from concourse.bass_utils import run_bass_kernel_spmd

F32 = mybir.dt.float32
BF16 = mybir.dt.bfloat16
I32 = mybir.dt.int32
NPBF = ml_dtypes.bfloat16

N_NODES = 10000
N_EDGES = 4096
IN_F = 300
OUT_F = 150
NCORES = 8
NPC = N_NODES // NCORES          # nodes per core (1250)
P = 128
NM = (NPC + P - 1) // P          # node chunks per core (10)
KCH = [(0, 128), (128, 128), (256, IN_F - 256)]  # contraction chunks of IN_F

LAST_RESULTS = None              # test harness reads exec_time from here


def _patch_tile_drain():
    """The stock Tile tail attaches every outstanding sem wait to one SP Drain
    instruction; this walrus build rejects multi-wait instructions ("Too many
    sync wait commands").  Split the waits onto individual SP nops instead."""
    if getattr(tile.TileContext, "_drain_split_patch", False):
        return

    def _drain_and_barrier(self, tick_clock, wait_clock):
        gc = tick_clock.global_clock
        vals = [int(x) for x in re.findall(r"-?\d+", repr(gc))]
        n = len(vals)
        for i, v in enumerate(vals):
            if v > 0:
                sub = bass_rust.VectorClock([v if j == i else 0 for j in range(n)])
                nop = self.nc.sync.nop()
                wait_clock.add_sem_waits(nop.ins, tile.ScopedClock({None: sub}))
        self.nc.sync.drain()
        self.nc.all_engine_barrier()
        popped = self.nc._tile_sem_poison_stack.pop()
        assert popped is self._sem_poison
        self.nc.clear_and_free_semaphores(list(self.sems.allocated().values()))
        self.nc.all_engine_barrier()

    tile.TileContext._drain_and_barrier = _drain_and_barrier
    tile.TileContext._drain_split_patch = True


def _build_program(C):
    """Emit the per-core Bass program for edge capacity C (multiple of 128)."""
    _patch_tile_drain()
    NE = C // P                  # edge chunks

    nc = bass.Bass()
    feats_d = nc.dram_tensor("features", [N_NODES, IN_F], F32, kind="ExternalInput")
    srcidx_d = nc.dram_tensor("srcidx", [C], I32, kind="ExternalInput")
    dstf_d = nc.dram_tensor("dstf", [C], F32, kind="ExternalInput")
    eff_d = nc.dram_tensor("eff", [C], F32, kind="ExternalInput")
    wbhi_d = nc.dram_tensor("wbhi", [2 * IN_F, OUT_F], BF16, kind="ExternalInput")
    wblo_d = nc.dram_tensor("wblo", [2 * IN_F, OUT_F], BF16, kind="ExternalInput")
    bias_d = nc.dram_tensor("bias1", [1, OUT_F], F32, kind="ExternalInput")
    identf_d = nc.dram_tensor("identf", [P, P], F32, kind="ExternalInput")
    identb_d = nc.dram_tensor("identb", [P, P], BF16, kind="ExternalInput")
    iota_d = nc.dram_tensor("iota", [P, NPC], F32, kind="ExternalInput")
    out_d = nc.dram_tensor("out", [NPC, OUT_F], F32, kind="ExternalOutput")

    with tile.TileContext(nc) as tc, ExitStack() as ctx:
        const = ctx.enter_context(tc.tile_pool(name="const", bufs=1))
        gath = ctx.enter_context(tc.tile_pool(name="gath", bufs=3))
        big = ctx.enter_context(tc.tile_pool(name="big", bufs=1))
        small = ctx.enter_context(tc.tile_pool(name="small", bufs=4))
        outp = ctx.enter_context(tc.tile_pool(name="outp", bufs=3))
        pt = ctx.enter_context(tc.tile_pool(name="pt", bufs=2, space="PSUM"))
        pq = ctx.enter_context(tc.tile_pool(name="pq", bufs=2, space="PSUM"))
        psums = ctx.enter_context(tc.tile_pool(name="psums", bufs=2, space="PSUM"))

        # ---- constants / small loads ----
        identf = const.tile([P, P], F32)
        nc.sync.dma_start(identf[:], identf_d[:])
        identb = const.tile([P, P], BF16)
        nc.sync.dma_start(identb[:], identb_d[:])
        iota_w = const.tile([P, NPC], F32)
        nc.sync.dma_start(iota_w[:], iota_d[:])
        bias_row = const.tile([1, OUT_F], F32)
        nc.sync.dma_start(bias_row[:], bias_d[:])
        dst_sb = const.tile([P, NE], F32)
        nc.sync.dma_start(dst_sb[:], dstf_d.rearrange("(c p) -> p c", p=P))
        eff_sb = const.tile([P, NE], F32)
        nc.sync.dma_start(eff_sb[:], eff_d.rearrange("(c p) -> p c", p=P))
        si_sb = const.tile([P, NE], I32)
        nc.sync.dma_start(si_sb[:], srcidx_d.rearrange("(c p) -> p c", p=P))
        wk = []  # (w0hi, w0lo, b0hi, b0lo) per K chunk
        for j, (o, w) in enumerate(KCH):
            t = []
            for nm, dram, off in (("w0h", wbhi_d, 0), ("w0l", wblo_d, 0),
                                  ("b0h", wbhi_d, IN_F), ("b0l", wblo_d, IN_F)):
                tt = const.tile([w, OUT_F], BF16, name=f"{nm}{j}", tag=f"{nm}{j}")
                nc.sync.dma_start(tt[:], dram[off + o:off + o + w, :])
                t.append(tt)
            wk.append(t)

        # ---- one-hot selection matrices S[e][p, n] = (dst[e*128+p] == n) ----
        s_full = []
        for e in range(NE):
            st = big.tile([P, NPC], BF16, name=f"sfull{e}", tag=f"sfull{e}")
            nc.vector.tensor_tensor(
                out=st[:], in0=dst_sb[:, e:e + 1].to_broadcast([P, NPC]),
                in1=iota_w[:], op=mybir.AluOpType.is_equal,
            )
            s_full.append(st)

        # ---- gather, hi/lo split, transpose ----
        hhT = [big.tile([w, C], BF16, name=f"hhT{j}", tag=f"hhT{j}")
               for j, (o, w) in enumerate(KCH)]
        hlT = [big.tile([w, C], BF16, name=f"hlT{j}", tag=f"hlT{j}")
               for j, (o, w) in enumerate(KCH)]
        for e in range(NE):
            hs = gath.tile([P, IN_F], F32, tag="hs")
            nc.gpsimd.indirect_dma_start(
                out=hs[:],
                out_offset=None,
                in_=feats_d[:],
                in_offset=bass.IndirectOffsetOnAxis(ap=si_sb[:, e:e + 1], axis=0),
            )
            hh = gath.tile([P, IN_F], BF16, tag="hh")
            nc.vector.tensor_copy(out=hh[:], in_=hs[:])
            hh32 = gath.tile([P, IN_F], F32, tag="hh32")
            nc.scalar.copy(out=hh32[:], in_=hh[:])
            hl = gath.tile([P, IN_F], BF16, tag="hl")
            nc.vector.tensor_tensor(out=hl[:], in0=hs[:], in1=hh32[:],
                                    op=mybir.AluOpType.subtract)
            for t, dstT in ((hh, hhT), (hl, hlT)):
                for j, (o, w) in enumerate(KCH):
                    tp = pt.tile([w, P], BF16, tag="tp")
                    nc.tensor.transpose(out=tp[:], in_=t[:, o:o + w],
                                        identity=identb[:])
                    eng = nc.scalar if j % 2 == 0 else nc.vector
                    if eng is nc.scalar:
                        nc.scalar.copy(out=dstT[j][:, e * P:(e + 1) * P], in_=tp[:])
                    else:
                        nc.vector.tensor_copy(out=dstT[j][:, e * P:(e + 1) * P],
                                              in_=tp[:])

        # ---- messages: msgs = eff * (h @ W0) + h @ B0, split to mhi+mlo ----
        mhis, mlos = [], []
        for e in range(NE):
            es = slice(e * P, (e + 1) * P)
            pp = pq.tile([P, OUT_F], F32, tag="pp")
            qp = pq.tile([P, OUT_F], F32, tag="qp")
            prods = ((hhT, 0), (hhT, 1), (hlT, 0))   # (hh@Whi, hh@Wlo, hl@Whi)
            for pi, (hT, wi) in enumerate(prods):
                for j in range(3):
                    nc.tensor.matmul(pp[:], lhsT=hT[j][:, es], rhs=wk[j][wi][:],
                                     start=(pi == 0 and j == 0),
                                     stop=(pi == 2 and j == 2))
            for pi, (hT, wi) in enumerate(prods):
                for j in range(3):
                    nc.tensor.matmul(qp[:], lhsT=hT[j][:, es], rhs=wk[j][2 + wi][:],
                                     start=(pi == 0 and j == 0),
                                     stop=(pi == 2 and j == 2))
            tmp = outp.tile([P, OUT_F], F32, tag="ptmp")
            nc.scalar.activation(tmp[:], pp[:], mybir.ActivationFunctionType.Copy,
                                 scale=eff_sb[:, e:e + 1])
            m32 = outp.tile([P, OUT_F], F32, tag="m32")
            nc.vector.tensor_tensor(out=m32[:], in0=tmp[:], in1=qp[:],
                                    op=mybir.AluOpType.add)
            mhi = big.tile([P, OUT_F + 1], BF16, name=f"mhi{e}", tag=f"mhi{e}")
            nc.vector.tensor_copy(out=mhi[:, 0:OUT_F], in_=m32[:])
            nc.gpsimd.memset(mhi[:, OUT_F:OUT_F + 1], 1.0)
            mhi32 = outp.tile([P, OUT_F], F32, tag="mhi32")
            nc.scalar.copy(out=mhi32[:], in_=mhi[:, 0:OUT_F])
            mlo = big.tile([P, OUT_F + 1], BF16, name=f"mlo{e}", tag=f"mlo{e}")
            nc.vector.tensor_tensor(out=mlo[:, 0:OUT_F], in0=m32[:], in1=mhi32[:],
                                    op=mybir.AluOpType.subtract)
            nc.gpsimd.memset(mlo[:, OUT_F:OUT_F + 1], 0.0)
            mhis.append(mhi)
            mlos.append(mlo)

        # ---- segment sum + mean/bias/relu epilogue, one node chunk at a time ----
        for m in range(NM):
            mo = m * P
            mw = min(P, NPC - mo)
            sp = psums.tile([mw, OUT_F + 1], F32, tag="sp")
            for e in range(NE):
                nc.tensor.matmul(sp[:], lhsT=s_full[e][:, mo:mo + mw],
                                 rhs=mhis[e][:],
                                 start=(e == 0), stop=False)
            for e in range(NE):
                nc.tensor.matmul(sp[:], lhsT=s_full[e][:, mo:mo + mw],
                                 rhs=mlos[e][:],
                                 start=False, stop=(e == NE - 1))
            cntm = small.tile([mw, 1], F32, tag="cntm")
            nc.vector.tensor_scalar_max(cntm[:], sp[:, OUT_F:OUT_F + 1], 1.0)
            recip = small.tile([mw, 1], F32, tag="recip")
            nc.vector.reciprocal(recip[:], cntm[:])
            ctp = pt.tile([1, mw], F32, tag="tp")
            nc.tensor.transpose(out=ctp[:], in_=cntm[:], identity=identf[0:mw, 0:mw])
            crow = small.tile([1, mw], F32, tag="crow")
            nc.scalar.copy(out=crow[:], in_=ctp[:])
            # sums += max(cnt,1) x bias  (rank-1), then relu(sums * 1/max(cnt,1))
            nc.tensor.matmul(sp[:, 0:OUT_F], lhsT=crow[:], rhs=bias_row[:],
                             start=False, stop=True, skip_group_check=True)
            ob = outp.tile([mw, OUT_F], F32, tag="ob")
            nc.scalar.activation(ob[:], sp[:, 0:OUT_F],
                                 mybir.ActivationFunctionType.Relu, scale=recip[:])
            nc.sync.dma_start(out_d[mo:mo + mw, :], ob[:])

    return nc


_MAX_WAITS = 1
_MAX_UPDATES = 1


def _split_syncs(nc):
    """This walrus build encodes only one sem wait/update slot per
    instruction.  Hoist excess waits onto preceding same-engine NoOps and
    defer excess updates onto trailing same-engine NoOps."""
    n = 0
    for f in nc.m.functions:
        for bb in f.blocks:
            out = []
            for inst in bb.instructions:
                si = getattr(inst, "sync_info", None)
                waits = list(si.on_wait) if si is not None and si.on_wait else []
                updates = list(si.on_update) if si is not None and si.on_update else []
                pre, post = [], []
                if len(waits) > _MAX_WAITS:
                    excess, waits = waits[:-_MAX_WAITS], waits[-_MAX_WAITS:]
                    for i in range(0, len(excess), _MAX_WAITS):
                        n += 1
                        pre.append(mybir.InstNoOp(
                            name=f"I-wsplit-{n}", engine=inst.engine, ins=[], outs=[],
                            sync_info=mybir.SyncInfo(
                                on_wait=excess[i:i + _MAX_WAITS], on_update=[]),
                        ))
                if len(updates) > _MAX_UPDATES:
                    updates, excess = updates[:_MAX_UPDATES], updates[_MAX_UPDATES:]
                    for i in range(0, len(excess), _MAX_UPDATES):
                        n += 1
                        post.append(mybir.InstNoOp(
                            name=f"I-usplit-{n}", engine=inst.engine, ins=[], outs=[],
                            sync_info=mybir.SyncInfo(
                                on_wait=[], on_update=excess[i:i + _MAX_UPDATES]),
                        ))
                if (pre or post) and si is not None:
                    inst.sync_info = mybir.SyncInfo(on_wait=waits, on_update=updates)
                out.extend(pre)
                out.append(inst)
                out.extend(post)
            bb.instructions[:] = out


_PROGRAM_CACHE = {}


def _get_program(C):
    if C not in _PROGRAM_CACHE:
        nc = _build_program(C)
        _split_syncs(nc)
        _PROGRAM_CACHE[C] = nc
    return _PROGRAM_CACHE[C]


def kernel(features, efeat, W_edge, b_edge, bias, src, dst):
    global LAST_RESULTS
    features = np.ascontiguousarray(np.asarray(features, np.float32))
    efeat = np.asarray(efeat, np.float32).reshape(-1)
    bias = np.asarray(bias, np.float32).reshape(-1)
    src = np.asarray(src, np.int32).reshape(-1)
    dst = np.asarray(dst, np.int32).reshape(-1)
    wb = np.concatenate(
        [np.asarray(W_edge, np.float32).reshape(IN_F, OUT_F),
         np.asarray(b_edge, np.float32).reshape(IN_F, OUT_F)], axis=0)
    wbhi = wb.astype(NPBF)
    wblo = (wb - wbhi.astype(np.float32)).astype(NPBF)

    core_of = dst // NPC
    sel = [np.nonzero(core_of == k)[0] for k in range(NCORES)]
    maxn = max(len(s) for s in sel)
    C = max(768, -(-maxn // P) * P)

    nc = _get_program(C)

    identf = np.eye(P, dtype=np.float32)
    identb = np.eye(P, dtype=NPBF)
    iota = np.tile(np.arange(NPC, dtype=np.float32), (P, 1))
    bias1 = np.ascontiguousarray(bias[None, :])

    in_maps = []
    for k in range(NCORES):
        s = sel[k]
        n = len(s)
        si = np.zeros((C,), np.int32)
        si[:n] = src[s]
        df = np.full((C,), -1.0, np.float32)
        df[:n] = (dst[s] - NPC * k).astype(np.float32)
        ef = np.zeros((C,), np.float32)
        ef[:n] = efeat[s]
        in_maps.append(dict(features=features, srcidx=si, dstf=df, eff=ef,
                            wbhi=wbhi, wblo=wblo, bias1=bias1,
                            identf=identf, identb=identb, iota=iota))

    res = run_bass_kernel_spmd(nc, in_maps, core_ids=list(range(NCORES)))
    LAST_RESULTS = res
    return np.concatenate([res.results[k]["out"] for k in range(NCORES)], axis=0)


# revision 10
# speedup vs baseline: 2.4057x; 1.2453x over previous
"""GNN message passing (nn_Net_16329465660089) as a Bass/Tile kernel on 8 trn2 cores.

Math:  We[e] = efeat[e] * W0 + B0  (rank-1 structure of the edge MLP), so
    msgs[e] = h_src[e] @ We[e] = efeat[e] * (h_src[e] @ W0) + h_src[e] @ B0
    out     = relu(segment_mean(msgs, dst) + bias)

Sharding: edges are routed to the core that owns their destination node
(core k owns nodes [1250k, 1250(k+1))), so each core computes its output
slice with no collectives.  Per core: gather h_src rows by indirect DMA,
hi/lo bf16 split + transpose on PE, shared matmuls (W0|B0 packed into one
rhs) + per-edge scale for the messages, one-hot matmul for the segment sum
(+ count), fused mean/bias/relu epilogue.  All matmuls run in bf16 with
hi/lo error compensation (h = hh + hl, W = Whi + Wlo, msgs = mhi + mlo),
which keeps the overall relative error at ~1e-5 while using full-rate bf16
PE passes.  The relu(mean + bias) is computed as
relu((sums + max(cnt,1)*bias) * (1/max(cnt,1))), exact for cnt==0 nodes.
"""

import os
import re
import numpy as np
from contextlib import ExitStack

import ml_dtypes
import bass_rust
import concourse.bass as bass
import concourse.mybir as mybir
import concourse.tile as tile
import concourse.bass_utils as bass_utils
from concourse.bass_utils import run_bass_kernel_spmd

F32 = mybir.dt.float32
BF16 = mybir.dt.bfloat16
FP16 = mybir.dt.float16
I32 = mybir.dt.int32
NPBF = ml_dtypes.bfloat16

N_NODES = 10000
N_EDGES = 4096
IN_F = 300
OUT_F = 150
NCORES = 8
NPC = N_NODES // NCORES          # nodes per core (1250)
P = 128
NM = (NPC + P - 1) // P          # node chunks per core (10)
KCH = [(0, 128), (128, 128), (256, IN_F - 256)]  # contraction chunks of IN_F

LAST_RESULTS = None              # test harness reads exec_time from here


def _patch_ldw_opt():
    """bir_verify_and_optimise hardcodes --enable-ldw-opt=false; flip it so
    walrus elides redundant LDWEIGHTS (we issue back-to-back matmuls with the
    same stationary operand)."""
    # walrus rejects explicit InstLdweights under ldw-opt; keep off unless
    # explicitly requested for experiments.
    if not os.environ.get("BASS_LDW_OPT") or getattr(bass_utils, "_ldw_patch", False):
        return
    orig = bass_utils.run_command

    def run_command_ldw(argv, **kw):
        argv = ["--enable-ldw-opt=true" if a == "--enable-ldw-opt=false" else a
                for a in argv]
        return orig(argv, **kw)

    bass_utils.run_command = run_command_ldw
    bass_utils._ldw_patch = True


def _patch_tile_drain():
    """The stock Tile tail attaches every outstanding sem wait to one SP Drain
    instruction; this walrus build rejects multi-wait instructions ("Too many
    sync wait commands").  Split the waits onto individual SP nops instead."""
    if getattr(tile.TileContext, "_drain_split_patch", False):
        return

    def _drain_and_barrier(self, tick_clock, wait_clock):
        gc = tick_clock.global_clock
        vals = [int(x) for x in re.findall(r"-?\d+", repr(gc))]
        n = len(vals)
        for i, v in enumerate(vals):
            if v > 0:
                sub = bass_rust.VectorClock([v if j == i else 0 for j in range(n)])
                nop = self.nc.sync.nop()
                wait_clock.add_sem_waits(nop.ins, tile.ScopedClock({None: sub}))
        self.nc.sync.drain()
        self.nc.all_engine_barrier()
        popped = self.nc._tile_sem_poison_stack.pop()
        assert popped is self._sem_poison
        self.nc.clear_and_free_semaphores(list(self.sems.allocated().values()))
        self.nc.all_engine_barrier()

    tile.TileContext._drain_and_barrier = _drain_and_barrier
    tile.TileContext._drain_split_patch = True


def _build_program(C):
    """Emit the per-core Bass program for edge capacity C (multiple of 128)."""
    _patch_tile_drain()
    _patch_ldw_opt()
    NE = C // P                  # edge chunks
    OF2 = 2 * OUT_F              # W0|B0 packed output width

    nc = bass.Bass()
    feats_d = nc.dram_tensor("features", [N_NODES, IN_F], F32, kind="ExternalInput")
    srcidx_d = nc.dram_tensor("srcidx", [C], I32, kind="ExternalInput")
    dsth_d = nc.dram_tensor("dsth", [C], FP16, kind="ExternalInput")
    eff_d = nc.dram_tensor("eff", [C], F32, kind="ExternalInput")
    wbhi_d = nc.dram_tensor("wbhi", [IN_F, OF2], BF16, kind="ExternalInput")
    wblo_d = nc.dram_tensor("wblo", [IN_F, OF2], BF16, kind="ExternalInput")
    biash_d = nc.dram_tensor("biash", [1, OUT_F], FP16, kind="ExternalInput")
    identb_d = nc.dram_tensor("identb", [P, P], BF16, kind="ExternalInput")
    identh_d = nc.dram_tensor("identh", [P, P], FP16, kind="ExternalInput")
    iota_d = nc.dram_tensor("iota", [P, NPC], FP16, kind="ExternalInput")
    out_d = nc.dram_tensor("out", [NPC, OUT_F], F32, kind="ExternalOutput")

    with tile.TileContext(nc) as tc, ExitStack() as ctx:
        const = ctx.enter_context(tc.tile_pool(name="const", bufs=1))
        gath = ctx.enter_context(tc.tile_pool(name="gath", bufs=3))
        big = ctx.enter_context(tc.tile_pool(name="big", bufs=1))
        small = ctx.enter_context(tc.tile_pool(name="small", bufs=4))
        outp = ctx.enter_context(tc.tile_pool(name="outp", bufs=3))
        pt = ctx.enter_context(tc.tile_pool(name="pt", bufs=2, space="PSUM"))
        pq = ctx.enter_context(tc.tile_pool(name="pq", bufs=3, space="PSUM"))
        psums = ctx.enter_context(tc.tile_pool(name="psums", bufs=3, space="PSUM"))

        # ---- constants / small loads ----
        identb = const.tile([P, P], BF16)
        nc.sync.dma_start(identb[:], identb_d[:])
        identh = const.tile([P, P], FP16)
        nc.sync.dma_start(identh[:], identh_d[:])
        iota_w = const.tile([P, NPC], FP16)
        nc.sync.dma_start(iota_w[:], iota_d[:])
        biash_row = const.tile([1, OUT_F], FP16)
        nc.sync.dma_start(biash_row[:], biash_d[:])
        dst_sb = const.tile([P, NE], FP16)
        nc.sync.dma_start(dst_sb[:], dsth_d.rearrange("(c p) -> p c", p=P))
        eff_sb = const.tile([P, NE], F32)
        nc.sync.dma_start(eff_sb[:], eff_d.rearrange("(c p) -> p c", p=P))
        si_sb = const.tile([P, NE], I32)
        nc.sync.dma_start(si_sb[:], srcidx_d.rearrange("(c p) -> p c", p=P))
        wk = []  # (whi, wlo) per K chunk, each [w, 2*OUT_F]
        for j, (o, w) in enumerate(KCH):
            th = const.tile([w, OF2], BF16, name=f"wh{j}", tag=f"wh{j}")
            nc.sync.dma_start(th[:], wbhi_d[o:o + w, :])
            tl = const.tile([w, OF2], BF16, name=f"wl{j}", tag=f"wl{j}")
            nc.sync.dma_start(tl[:], wblo_d[o:o + w, :])
            wk.append((th, tl))

        # ---- one-hot selection matrices S[e][p, n] = (dst[e*128+p] == n) ----
        s_full = []
        for e in range(NE):
            st = big.tile([P, NPC], BF16, name=f"sfull{e}", tag=f"sfull{e}")
            nc.vector.tensor_tensor(
                out=st[:], in0=dst_sb[:, e:e + 1].to_broadcast([P, NPC]),
                in1=iota_w[:], op=mybir.AluOpType.is_equal,
            )
            s_full.append(st)

        # ---- gather, hi/lo split, transpose ----
        hhT = [big.tile([w, C], BF16, name=f"hhT{j}", tag=f"hhT{j}")
               for j, (o, w) in enumerate(KCH)]
        hlT = [big.tile([w, C], BF16, name=f"hlT{j}", tag=f"hlT{j}")
               for j, (o, w) in enumerate(KCH)]
        ncopy = 0
        for e in range(NE):
            hs = gath.tile([P, IN_F], F32, tag="hs")
            nc.gpsimd.indirect_dma_start(
                out=hs[:],
                out_offset=None,
                in_=feats_d[:],
                in_offset=bass.IndirectOffsetOnAxis(ap=si_sb[:, e:e + 1], axis=0),
            )
            hh = gath.tile([P, IN_F], BF16, tag="hh")
            nc.vector.tensor_copy(out=hh[:], in_=hs[:])
            hh32 = gath.tile([P, IN_F], F32, tag="hh32")
            nc.scalar.copy(out=hh32[:], in_=hh[:])
            hl = gath.tile([P, IN_F], BF16, tag="hl")
            nc.vector.tensor_tensor(out=hl[:], in0=hs[:], in1=hh32[:],
                                    op=mybir.AluOpType.subtract)
            for t, dstT in ((hh, hhT), (hl, hlT)):
                for j, (o, w) in enumerate(KCH):
                    tp = pt.tile([w, P], BF16, tag="tp")
                    nc.tensor.transpose(out=tp[:], in_=t[:, o:o + w],
                                        identity=identb[:])
                    ncopy += 1
                    if ncopy % 2 == 0:
                        nc.scalar.copy(out=dstT[j][:, e * P:(e + 1) * P], in_=tp[:])
                    else:
                        nc.vector.tensor_copy(out=dstT[j][:, e * P:(e + 1) * P],
                                              in_=tp[:])

        # ---- messages: msgs = eff * (h @ W0) + h @ B0, split to mhi+mlo ----
        mhis, mlos = [], []
        prods = ((hhT, 0), (hhT, 1), (hlT, 0))   # (hh@Whi, hh@Wlo, hl@Whi)
        for e in range(NE):
            es = slice(e * P, (e + 1) * P)
            pqt = pq.tile([P, OF2], F32, tag="pq")
            n = 0
            for hT, wi in prods:
                for j in range(3):
                    nc.tensor.matmul(pqt[:], lhsT=hT[j][:, es], rhs=wk[j][wi][:],
                                     start=(n == 0), stop=(n == 8))
                    n += 1
            tmp = outp.tile([P, OUT_F], F32, tag="ptmp")
            nc.scalar.activation(tmp[:], pqt[:, 0:OUT_F],
                                 mybir.ActivationFunctionType.Copy,
                                 scale=eff_sb[:, e:e + 1])
            m32 = outp.tile([P, OUT_F], F32, tag="m32")
            nc.vector.tensor_tensor(out=m32[:], in0=tmp[:], in1=pqt[:, OUT_F:OF2],
                                    op=mybir.AluOpType.add)
            mhi = big.tile([P, OUT_F + 1], BF16, name=f"mhi{e}", tag=f"mhi{e}")
            nc.vector.tensor_copy(out=mhi[:, 0:OUT_F], in_=m32[:])
            nc.gpsimd.memset(mhi[:, OUT_F:OUT_F + 1], 1.0)
            mhi32 = outp.tile([P, OUT_F], F32, tag="mhi32")
            nc.scalar.copy(out=mhi32[:], in_=mhi[:, 0:OUT_F])
            mlo = big.tile([P, OUT_F + 1], BF16, name=f"mlo{e}", tag=f"mlo{e}")
            nc.vector.tensor_tensor(out=mlo[:, 0:OUT_F], in0=m32[:], in1=mhi32[:],
                                    op=mybir.AluOpType.subtract)
            nc.gpsimd.memset(mlo[:, OUT_F:OUT_F + 1], 0.0)
            mhis.append(mhi)
            mlos.append(mlo)

        # ---- segment sum + mean/bias/relu epilogue, one node chunk at a time ----
        for m in range(NM):
            mo = m * P
            mw = min(P, NPC - mo)
            sp = psums.tile([mw, OUT_F + 1], F32, tag="sp")
            n = 0
            for e in range(NE):
                # adjacent same-lhsT pair lets ldw-opt elide the second load
                nc.tensor.matmul(sp[:], lhsT=s_full[e][:, mo:mo + mw],
                                 rhs=mhis[e][:], start=(n == 0), stop=False)
                n += 1
                nc.tensor.matmul(sp[:], lhsT=s_full[e][:, mo:mo + mw],
                                 rhs=mlos[e][:], start=False,
                                 stop=(n == 2 * NE - 1))
                n += 1
            cntm = small.tile([mw, 1], FP16, tag="cntm")
            nc.vector.tensor_scalar_max(cntm[:], sp[:, OUT_F:OUT_F + 1], 1.0)
            recip = small.tile([mw, 1], F32, tag="recip")
            nc.vector.reciprocal(recip[:], cntm[:])
            ctp = pt.tile([1, mw], FP16, tag="tp")
            nc.tensor.transpose(out=ctp[:], in_=cntm[:], identity=identh[0:mw, 0:mw])
            crow = small.tile([1, mw], FP16, tag="crow")
            nc.scalar.copy(out=crow[:], in_=ctp[:])
            # sums += max(cnt,1) x bias  (rank-1), then relu(sums * 1/max(cnt,1))
            nc.tensor.matmul(sp[:, 0:OUT_F], lhsT=crow[:], rhs=biash_row[:],
                             start=False, stop=True, skip_group_check=True)
            ob = outp.tile([mw, OUT_F], F32, tag="ob")
            nc.scalar.activation(ob[:], sp[:, 0:OUT_F],
                                 mybir.ActivationFunctionType.Relu, scale=recip[:])
            nc.sync.dma_start(out_d[mo:mo + mw, :], ob[:])

    return nc


_MAX_WAITS = 1
_MAX_UPDATES = 1


def _split_syncs(nc):
    """This walrus build encodes only one sem wait/update slot per
    instruction.  Hoist excess waits onto preceding same-engine NoOps and
    defer excess updates onto trailing same-engine NoOps."""
    n = 0
    for f in nc.m.functions:
        for bb in f.blocks:
            out = []
            for inst in bb.instructions:
                si = getattr(inst, "sync_info", None)
                waits = list(si.on_wait) if si is not None and si.on_wait else []
                updates = list(si.on_update) if si is not None and si.on_update else []
                pre, post = [], []
                if len(waits) > _MAX_WAITS:
                    excess, waits = waits[:-_MAX_WAITS], waits[-_MAX_WAITS:]
                    for i in range(0, len(excess), _MAX_WAITS):
                        n += 1
                        pre.append(mybir.InstNoOp(
                            name=f"I-wsplit-{n}", engine=inst.engine, ins=[], outs=[],
                            sync_info=mybir.SyncInfo(
                                on_wait=excess[i:i + _MAX_WAITS], on_update=[]),
                        ))
                if len(updates) > _MAX_UPDATES:
                    updates, excess = updates[:_MAX_UPDATES], updates[_MAX_UPDATES:]
                    for i in range(0, len(excess), _MAX_UPDATES):
                        n += 1
                        post.append(mybir.InstNoOp(
                            name=f"I-usplit-{n}", engine=inst.engine, ins=[], outs=[],
                            sync_info=mybir.SyncInfo(
                                on_wait=[], on_update=excess[i:i + _MAX_UPDATES]),
                        ))
                if (pre or post) and si is not None:
                    inst.sync_info = mybir.SyncInfo(on_wait=waits, on_update=updates)
                out.extend(pre)
                out.append(inst)
                out.extend(post)
            bb.instructions[:] = out


_PROGRAM_CACHE = {}


def _get_program(C):
    if C not in _PROGRAM_CACHE:
        nc = _build_program(C)
        _split_syncs(nc)
        _PROGRAM_CACHE[C] = nc
    return _PROGRAM_CACHE[C]


def kernel(features, efeat, W_edge, b_edge, bias, src, dst):
    global LAST_RESULTS
    features = np.ascontiguousarray(np.asarray(features, np.float32))
    efeat = np.asarray(efeat, np.float32).reshape(-1)
    bias = np.asarray(bias, np.float32).reshape(-1)
    src = np.asarray(src, np.int32).reshape(-1)
    dst = np.asarray(dst, np.int32).reshape(-1)
    wb = np.concatenate(
        [np.asarray(W_edge, np.float32).reshape(IN_F, OUT_F),
         np.asarray(b_edge, np.float32).reshape(IN_F, OUT_F)], axis=1)
    wbhi = wb.astype(NPBF)
    wblo = (wb - wbhi.astype(np.float32)).astype(NPBF)

    core_of = dst // NPC
    sel = [np.nonzero(core_of == k)[0] for k in range(NCORES)]
    maxn = max(len(s) for s in sel)
    C = max(640, -(-maxn // P) * P)

    nc = _get_program(C)

    identb = np.eye(P, dtype=NPBF)
    identh = np.eye(P, dtype=np.float16)
    iota = np.tile(np.arange(NPC, dtype=np.float16), (P, 1))
    biash = np.ascontiguousarray(bias[None, :].astype(np.float16))

    in_maps = []
    for k in range(NCORES):
        s = sel[k]
        n = len(s)
        si = np.zeros((C,), np.int32)
        si[:n] = src[s]
        df = np.full((C,), -1.0, np.float16)
        df[:n] = (dst[s] - NPC * k).astype(np.float16)
        ef = np.zeros((C,), np.float32)
        ef[:n] = efeat[s]
        in_maps.append(dict(features=features, srcidx=si, dsth=df, eff=ef,
                            wbhi=wbhi, wblo=wblo, biash=biash,
                            identb=identb, identh=identh, iota=iota))

    res = run_bass_kernel_spmd(nc, in_maps, core_ids=list(range(NCORES)))
    LAST_RESULTS = res
    return np.concatenate([res.results[k]["out"] for k in range(NCORES)], axis=0)
